# revision 24
# baseline (speedup 1.0000x reference)
"""Trainium2 Bass kernel for nn_CausalSelfAttention_72653666779352.

Sharding: 8 cores = 2 batches x 4 kv-groups. Core (b, g) owns
global kv head E=g (q heads 2g, 2g+1) and local kv head F=4+g
(q heads 8+2g, 9+2g). All device compute is in transposed layout
(feature dims on partitions, time on free axis), bf16 matmuls
(FWL weight loads + 1 cycle/row streaming; fp32r would disable FWL).
Scalar engine uses ONE activation table set (exp+ln): sigmoid is
computed as 1/(1+e^-x) and rsqrt as exp(-0.5*ln(x)) to avoid
ACT_TABLE_LOAD thrash. c_proj is fused into the attention loop
per q-chunk, reusing freed PSUM y-banks. Row-parallel partials
([C, T] bf16 per core) are summed on the host (unshard).
"""
import contextlib
import numpy as np
import ml_dtypes

BF = ml_dtypes.bfloat16
B, T, C = 2, 2048, 1024
NH, NKV = 16, 8
HD = 64
VGC = 32
TQC = 512            # tq chunk width
NQC = T // TQC       # 4
NKB = T // 128       # 16
EPS = float(np.finfo(np.float32).eps)
SCALE = 1.0 / 8.0    # 1/sqrt(HD)

_CACHE = {}


def _ranges(qc, pair, window):
    """kb tiles for (pair, qc): list of (kb, lo, hi, band_lo, mtype)."""
    out = []
    for kb in range(NKB):
        o = 128 * kb - TQC * qc
        lo = max(0, o)
        hi = TQC if pair == 0 else min(TQC, o + window + 128)
        if lo >= hi:
            continue
        cband = o if 0 <= o < TQC else None
        wband = None
        if pair == 1:
            wb = o + window
            if 0 <= wb < TQC:
                wband = wb
        assert not (cband is not None and wband is not None)
        if cband is not None:
            out.append((kb, lo, hi, cband, "c"))
        elif wband is not None:
            out.append((kb, lo, hi, wband, "w"))
        else:
            out.append((kb, lo, hi, None, None))
    # first tile must cover the full [0, TQC) col range (PSUM has_written)
    first = next(i for i, r in enumerate(out) if r[1] == 0 and r[2] == TQC)
    out[0], out[first] = out[first], out[0]
    return out


def _build(window):
    import concourse.mybir as mybir
    import concourse.tile as tile
    from concourse import bacc

    f32, bf16 = mybir.dt.float32, mybir.dt.bfloat16
    nc = bacc.Bacc("TRN2", target_bir_lowering=False, debug=False)

    def din(name, shape, dt=bf16):
        return nc.dram_tensor(name, shape, dt, kind="ExternalInput").ap()

    xT_d = din("xT", [C, T])
    wq_d = din("wq", [C, 256])
    wk_d = din("wk", [C, 128])
    wv_d = din("wv", [C, 128])
    wgate_d = din("wgate", [VGC, 128])
    wproj_d = din("wproj", [256, C])
    m1_d = din("m1", [128, 128])        # block-diag 32x32 ones (q rms sums)
    mk_d = din("mk", [128, 64])         # k rms indicator
    ident_d = din("ident", [128, 128])
    veT_d = din("veT", [128, T])        # 2*ve, per-core heads, transposed
    cos4_d = din("cos4", [128, T])
    sin4_d = din("sin4", [128, T])
    cmask_d = din("cmask", [128, 256])
    wmask_d = din("wmask", [128, 256])
    outT_d = nc.dram_tensor("outT", [C, T], bf16, kind="ExternalOutput").ap()

    EXP = mybir.ActivationFunctionType.Exp
    TANH = mybir.ActivationFunctionType.Tanh
    SQRT = mybir.ActivationFunctionType.Sqrt

    with tile.TileContext(nc) as tc, contextlib.ExitStack() as top:
        pers = top.enter_context(tc.tile_pool(name="pers", bufs=1))
        vap = top.enter_context(tc.tile_pool(name="vap", bufs=1))
        yTp = top.enter_context(tc.tile_pool(name="yTp", bufs=1))

        # ---- persistent loads (weights split per contraction chunk) ----
        wq_sb = pers.tile([128, 8, 256], bf16)
        wk_sb = pers.tile([128, 8, 128], bf16)
        wv_sb = pers.tile([128, 8, 128], bf16)
        wq_r = wq_d.rearrange("(a p) m -> p a m", p=128)
        wk_r = wk_d.rearrange("(a p) m -> p a m", p=128)
        wv_r = wv_d.rearrange("(a p) m -> p a m", p=128)
        for ct in range(8):
            nc.sync.dma_start(out=wq_sb[:, ct, :], in_=wq_r[:, ct, :])
            nc.sync.dma_start(out=wk_sb[:, ct, :], in_=wk_r[:, ct, :])
            nc.sync.dma_start(out=wv_sb[:, ct, :], in_=wv_r[:, ct, :])
        wgate_sb = pers.tile([VGC, 128], bf16)
        nc.sync.dma_start(out=wgate_sb, in_=wgate_d)
        eps_sb = pers.tile([128, 1], f32)
        nc.vector.memset(eps_sb, EPS)

        # persistent activations
        qf1 = pers.tile([128, T], bf16)   # [A | B] per-head normed q
        qf2 = pers.tile([128, T], bf16)   # [C | D]
        k_g = pers.tile([128, T], bf16)   # [E | E]
        k_l = pers.tile([128, T], bf16)   # [F | F]
        vT_sb = pers.tile([128, T], bf16)  # [E dims | F dims]
        veT_sb = pers.tile([128, T], bf16)
        cos4_sb = pers.tile([128, T], bf16)
        sin4_sb = pers.tile([128, T], bf16)
        yT_g = yTp.tile([128, T], bf16)
        yT_l = yTp.tile([128, T], bf16)
        gth = pers.tile([128, T], bf16)   # tanh(g/2); gate = 1 + gth
        gx = pers.tile([VGC, T], bf16)    # x[:, 0:32] transposed, for gate
        nc.sync.dma_start(out=gx, in_=xT_d[0:VGC, :])

        # va tiles: [v(64) | ones(64)]; ones prefilled up-front on gpsimd
        va = {}
        for kb in range(NKB):
            for h in range(2):
                t = vap.tile([128, 128], bf16, name=f"va{h}_{kb}")
                nc.gpsimd.memset(t[:, 64:128], 1.0)
                va[(h, kb)] = t

        # deferred constant loads (behind the critical first-chunk path)
        m1_sb = pers.tile([128, 128], bf16)
        mk_sb = pers.tile([128, 64], bf16)
        ident_sb = pers.tile([128, 128], bf16)
        cmask_sb = pers.tile([128, 2, 128], bf16)
        wmask_sb = pers.tile([128, 2, 128], bf16)
        wp_sb = pers.tile([128, 2, C], bf16)

        # ---- phase 1: projections + rope + rmsnorm + gate/ve + v transp ----
        with contextlib.ExitStack() as ph1:
            p1s = ph1.enter_context(tc.tile_pool(name="p1s", bufs=2))
            psA = ph1.enter_context(tc.tile_pool(name="psA", bufs=5, space="PSUM"))
            psB = ph1.enter_context(tc.tile_pool(name="psB", bufs=2, space="PSUM"))
            psV = ph1.enter_context(tc.tile_pool(name="psV", bufs=1, space="PSUM"))

            xT_r = xT_d.rearrange("(a p) t -> p a t", p=128)
            xc0 = []
            for ct in range(8):
                xt = p1s.tile([128, TQC], bf16, tag=f"x{ct}", name=f"x{ct}_0", bufs=2)
                nc.sync.dma_start(out=xt, in_=xT_r[:, ct, 0:TQC])
                xc0.append(xt)

            nc.sync.dma_start(out=cos4_sb, in_=cos4_d)
            nc.sync.dma_start(out=sin4_sb, in_=sin4_d)
            nc.sync.dma_start(out=veT_sb, in_=veT_d)
            nc.sync.dma_start(out=m1_sb, in_=m1_d)
            nc.sync.dma_start(out=mk_sb, in_=mk_d)
            nc.sync.dma_start(out=ident_sb, in_=ident_d)
            nc.sync.dma_start(out=cmask_sb, in_=cmask_d.rearrange("p (a c) -> p a c", a=2))
            nc.sync.dma_start(out=wmask_sb, in_=wmask_d.rearrange("p (a c) -> p a c", a=2))
            nc.sync.dma_start(out=wp_sb, in_=wproj_d.rearrange("(a p) m -> p a m", p=128))

            # gate for all chunks up-front: one tanh batch = one exp-set
            # table residency; phase-1 ACT then only needs sqrt+copy.
            for qc in range(NQC):
                gts = slice(qc * TQC, (qc + 1) * TQC)
                g_ps = psB.tile([128, TQC], f32, tag="rms", name=f"g_{qc}")
                nc.tensor.matmul(g_ps, wgate_sb, gx[:, gts], start=True, stop=True)
                nc.scalar.activation(gth[:, gts], g_ps, TANH, bias=0.0, scale=0.5)

            for qc in range(NQC):
                ts = slice(qc * TQC, (qc + 1) * TQC)
                if qc == 0:
                    xc = xc0
                else:
                    xc = []
                    for ct in range(8):
                        xt = p1s.tile([128, TQC], bf16, tag=f"x{ct}", name=f"x{ct}_{qc}",
                                      bufs=2)
                        nc.sync.dma_start(out=xt, in_=xT_r[:, ct, ts])
                        xc.append(xt)

                qlo_ps = psA.tile([128, TQC], f32, tag="pj", name=f"qlo_{qc}")
                qhi_ps = psA.tile([128, TQC], f32, tag="pj", name=f"qhi_{qc}")
                k_ps = psA.tile([128, TQC], f32, tag="pj", name=f"k_{qc}")
                v_ps = psA.tile([128, TQC], f32, tag="pj", name=f"v_{qc}")
                for ct in range(8):
                    st, sp = (ct == 0), (ct == 7)
                    nc.tensor.matmul(v_ps, wv_sb[:, ct, :], xc[ct], start=st, stop=sp)
                    nc.tensor.matmul(k_ps, wk_sb[:, ct, :], xc[ct], start=st, stop=sp)
                    nc.tensor.matmul(qlo_ps, wq_sb[:, ct, 0:128], xc[ct], start=st, stop=sp)
                    nc.tensor.matmul(qhi_ps, wq_sb[:, ct, 128:256], xc[ct], start=st, stop=sp)

                # gate: v += (1 + tanh(g/2)) * ve   (== v + 2*sigmoid(g)*ve)
                gv = p1s.tile([128, TQC], bf16, tag="gv", name=f"gv_{qc}")
                nc.vector.scalar_tensor_tensor(
                    gv, gth[:, ts], 1.0, veT_sb[:, ts],
                    op0=mybir.AluOpType.add, op1=mybir.AluOpType.mult)
                nc.vector.tensor_add(vT_sb[:, ts], v_ps, gv)

                # v transpose + va assembly for this chunk's 4 k-blocks
                for kb in range(4 * qc, 4 * qc + 4):
                    vt_ps = psV.tile([128, 128], bf16, tag="vt", name=f"vt_{kb}")
                    nc.tensor.transpose(vt_ps, vT_sb[:, kb * 128:(kb + 1) * 128],
                                        ident_sb)
                    for h in range(2):
                        nc.scalar.copy(va[(h, kb)][:, 0:64],
                                       vt_ps[:, h * 64:(h + 1) * 64])

                # evacuate q/k PSUM to bf16 (ACT) so rope runs in DVE 2x mode
                qloc = p1s.tile([128, TQC], bf16, tag="qloc", name=f"qloc_{qc}")
                qhic = p1s.tile([128, TQC], bf16, tag="qhic", name=f"qhic_{qc}")
                kc = p1s.tile([128, TQC], bf16, tag="kc", name=f"kc_{qc}")
                nc.scalar.copy(qloc, qlo_ps)
                nc.scalar.copy(qhic, qhi_ps)
                nc.scalar.copy(kc, k_ps)

                # q rope (split layout)
                mc = p1s.tile([128, TQC], bf16, tag="mc", name=f"mc_{qc}")
                ms = p1s.tile([128, TQC], bf16, tag="ms", name=f"ms_{qc}")
                rl = p1s.tile([128, TQC], bf16, tag="rl", name=f"rl_{qc}")
                rh = p1s.tile([128, TQC], bf16, tag="rh", name=f"rh_{qc}")
                nc.vector.tensor_mul(mc, qloc, cos4_sb[:, ts])
                nc.vector.tensor_mul(ms, qhic, sin4_sb[:, ts])
                nc.vector.tensor_add(rl, mc, ms)
                nc.vector.tensor_mul(mc, qhic, cos4_sb[:, ts])
                nc.vector.tensor_mul(ms, qloc, sin4_sb[:, ts])
                nc.vector.tensor_sub(rh, mc, ms)

                # k rope: kc rows [Elo|Flo|Ehi|Fhi] -> kr same row order
                kr = p1s.tile([128, TQC], bf16, tag="kr", name=f"kr_{qc}")
                mck = p1s.tile([64, TQC], bf16, tag="mck", name=f"mck_{qc}", bufs=1)
                msk = p1s.tile([64, TQC], bf16, tag="msk", name=f"msk_{qc}", bufs=1)
                nc.vector.tensor_mul(mck, kc[0:64, :], cos4_sb[0:64, ts])
                nc.vector.tensor_mul(msk, kc[64:128, :], sin4_sb[64:128, ts])
                nc.vector.tensor_add(kr[0:64, :], mck, msk)
                nc.vector.tensor_mul(mck, kc[64:128, :], cos4_sb[64:128, ts])
                nc.vector.tensor_mul(msk, kc[0:64, :], sin4_sb[0:64, ts])
                nc.vector.tensor_sub(kr[64:128, :], mck, msk)

                # rmsnorm q
                q2a = p1s.tile([128, TQC], bf16, tag="q2a", name=f"q2a_{qc}")
                q2b = p1s.tile([128, TQC], bf16, tag="q2b", name=f"q2b_{qc}")
                nc.vector.tensor_mul(q2a, rl, rl)
                nc.vector.tensor_mul(q2b, rh, rh)
                rms_q = psB.tile([128, TQC], f32, tag="rms", name=f"rmsq_{qc}")
                nc.tensor.matmul(rms_q, m1_sb, q2a, start=True, stop=False)
                nc.tensor.matmul(rms_q, m1_sb, q2b, start=False, stop=True)
                sq_sb = p1s.tile([128, TQC], f32, tag="sq", name=f"sq_{qc}")
                nc.scalar.activation(sq_sb, rms_q, SQRT, bias=eps_sb, scale=1.0 / HD)
                rq = p1s.tile([128, TQC], f32, tag="rq", name=f"rq_{qc}")
                nc.vector.reciprocal_approx_fast(rq, sq_sb)
                qn_lo = p1s.tile([128, TQC], bf16, tag="qnl", name=f"qnl_{qc}", bufs=2)
                qn_hi = p1s.tile([128, TQC], bf16, tag="qnh", name=f"qnh_{qc}", bufs=2)
                nc.vector.tensor_mul(qn_lo, rl, rq)
                nc.vector.tensor_mul(qn_hi, rh, rq)

                # rmsnorm k
                k2 = p1s.tile([128, TQC], bf16, tag="k2", name=f"k2_{qc}")
                nc.vector.tensor_mul(k2, kr, kr)
                rms_k = psB.tile([64, TQC], f32, tag="rms", name=f"rmsk_{qc}")
                nc.tensor.matmul(rms_k, mk_sb, k2, start=True, stop=True)
                sk_sb = p1s.tile([64, TQC], f32, tag="sk", name=f"sk_{qc}", bufs=1)
                nc.scalar.activation(sk_sb, rms_k, SQRT, bias=eps_sb[0:64, :], scale=1.0 / HD)
                rk = p1s.tile([64, TQC], f32, tag="rk", name=f"rk_{qc}", bufs=1)
                nc.vector.reciprocal_approx_fast(rk, sk_sb)
                rkd = p1s.tile([128, TQC], f32, tag="rkd", name=f"rkd_{qc}")
                nc.scalar.copy(rkd[0:64, :], rk)
                nc.scalar.copy(rkd[64:128, :], rk)
                kn = p1s.tile([128, TQC], bf16, tag="kn", name=f"kn_{qc}", bufs=2)
                nc.vector.tensor_mul(kn, kr, rkd)

                # permute split layout -> per-head tiles (SBUF->SBUF DMA)
                for i in range(4):
                    dst = qf1 if i < 2 else qf2
                    base = (i % 2) * 64
                    nc.sync.dma_start(out=dst[base:base + 32, ts],
                                      in_=qn_lo[i * 32:(i + 1) * 32, :])
                    nc.sync.dma_start(out=dst[base + 32:base + 64, ts],
                                      in_=qn_hi[i * 32:(i + 1) * 32, :])
                for half in range(2):
                    b0 = half * 64
                    nc.sync.dma_start(out=k_g[b0:b0 + 32, ts], in_=kn[0:32, :])
                    nc.sync.dma_start(out=k_g[b0 + 32:b0 + 64, ts], in_=kn[64:96, :])
                    nc.sync.dma_start(out=k_l[b0:b0 + 32, ts], in_=kn[32:64, :])
                    nc.sync.dma_start(out=k_l[b0 + 32:b0 + 64, ts], in_=kn[96:128, :])

        # ---- phase 3: attention + fused c_proj per q-chunk ----
        with contextlib.ExitStack() as ph3:
            psS = ph3.enter_context(tc.tile_pool(name="psS", bufs=1, space="PSUM"))
            psY = ph3.enter_context(tc.tile_pool(name="psY", bufs=2, space="PSUM"))
            wk3 = ph3.enter_context(tc.tile_pool(name="wk3", bufs=3))

            cfg = [(qf1, k_g, yT_g), (qf2, k_l, yT_l)]
            for qc in range(NQC):
                ts = slice(qc * TQC, (qc + 1) * TQC)
                rrs = [_ranges(qc, pair, window) for pair in range(2)]
                yps = [[psY.tile([128, TQC], f32, tag=f"y{pair}{h}", bufs=1,
                                 name=f"y{pair}{h}_{qc}")
                        for h in range(2)] for pair in range(2)]
                nmax = max(len(r) for r in rrs)
                for idx in range(nmax):
                    for pair in range(2):
                        if idx >= len(rrs[pair]):
                            continue
                        qf, kt, yT = cfg[pair]
                        kb, lo, hi, band, mt = rrs[pair][idx]
                        s2 = psS.tile([128, 2, TQC], f32, tag=f"s{pair}",
                                      name=f"s{pair}_{qc}_{kb}")
                        ks = slice(kb * 128, (kb + 1) * 128)
                        qs = slice(qc * TQC + lo, qc * TQC + hi)
                        nc.tensor.matmul(s2[:, 0, lo:hi], kt[0:64, ks], qf[0:64, qs],
                                         start=True, stop=True, tile_position=(0, 0))
                        nc.tensor.matmul(s2[:, 1, lo:hi], kt[64:128, ks], qf[64:128, qs],
                                         start=True, stop=True, tile_position=(64, 0))
                        st, sp = (idx == 0), (idx == len(rrs[pair]) - 1)
                        e2 = wk3.tile([128, 2, TQC], bf16, tag=f"e{pair}",
                                      name=f"e{pair}_{qc}_{kb}")
                        nc.scalar.activation(e2[:, :, lo:hi], s2[:, :, lo:hi], EXP,
                                             bias=0.0, scale=SCALE)
                        if band is not None:
                            msk_t = cmask_sb if mt == "c" else wmask_sb
                            nc.vector.tensor_mul(e2[:, :, band:band + 128],
                                                 e2[:, :, band:band + 128], msk_t)
                        for h in range(2):
                            nc.tensor.matmul(yps[pair][h][:, lo:hi], va[(pair, kb)],
                                             e2[:, h, lo:hi], start=st, stop=sp)
                for pair in range(2):
                    qf, kt, yT = cfg[pair]
                    for h in range(2):
                        y_ps = yps[pair][h]
                        den = wk3.tile([64, TQC], f32, tag=f"dn{pair}{h}",
                                       name=f"dn{pair}{h}_{qc}")
                        nc.vector.tensor_copy(den, y_ps[64:128, :])
                        rec = wk3.tile([64, TQC], f32, tag=f"rec{pair}{h}",
                                       name=f"rec{pair}{h}_{qc}")
                        nc.vector.reciprocal_approx_fast(rec, den)
                        nc.vector.tensor_mul(yT[h * 64:(h + 1) * 64, ts],
                                             y_ps[0:64, :], rec)

                # fused c_proj for this chunk (reuses pair-0 y banks)
                for cb in range(8):
                    cs = slice(cb * 128, (cb + 1) * 128)
                    pj = psY.tile([128, TQC], f32, tag=f"y0{cb % 2}", bufs=1,
                                  name=f"pj_{cb}_{qc}")
                    nc.tensor.matmul(pj, wp_sb[:, 0, cs], yT_g[:, ts],
                                     start=True, stop=False)
                    nc.tensor.matmul(pj, wp_sb[:, 1, cs], yT_l[:, ts],
                                     start=False, stop=True)
                    ot = wk3.tile([128, TQC], bf16, tag=f"ot{cb % 4}",
                                  name=f"ot_{cb}_{qc}")
                    if cb % 2 == 0:
                        nc.vector.tensor_copy(ot, pj)
                    else:
                        nc.scalar.copy(ot, pj)
                    nc.sync.dma_start(out=outT_d[cs, ts], in_=ot)

    nc.compile()
    return nc


def _host_inputs(x, ve, cos, sin, Wq, Wk, Wv, Wproj, Wgate):
    """Per-core input maps (core = b*4 + g)."""
    x = np.asarray(x, np.float32)
    ve = np.asarray(ve, np.float32)
    cos = np.asarray(cos, np.float32).reshape(T, -1)   # [T, 32]
    sin = np.asarray(sin, np.float32).reshape(T, -1)
    Wq = np.asarray(Wq, np.float32)
    Wk = np.asarray(Wk, np.float32)
    Wv = np.asarray(Wv, np.float32)
    Wproj = np.asarray(Wproj, np.float32)
    Wgate = np.asarray(Wgate, np.float32)

    cos4 = np.ascontiguousarray(np.tile(cos.T, (4, 1))).astype(BF)  # [128, T]
    sin4 = np.ascontiguousarray(np.tile(sin.T, (4, 1))).astype(BF)
    ident = np.eye(128, dtype=np.float32).astype(BF)
    m1 = np.kron(np.eye(4, dtype=np.float32), np.ones((32, 32), np.float32)).astype(BF)
    mk = np.zeros((128, 64), np.float32)
    for p in range(128):
        h = (p % 64) // 32
        mk[p, h * 32:(h + 1) * 32] = 1.0
    mk = mk.astype(BF)
    pp = np.arange(128)[:, None]
    cc = np.arange(128)[None, :]
    cmask = np.tile((cc >= pp).astype(np.float32), (1, 2)).reshape(128, 256).astype(BF)
    wmask = np.tile((cc <= pp).astype(np.float32), (1, 2)).reshape(128, 256).astype(BF)

    xT = [np.ascontiguousarray(x[b].T).astype(BF) for b in range(B)]
    maps = []
    for core in range(8):
        b, g = divmod(core, 4)
        heads = [2 * g, 2 * g + 1, 8 + 2 * g, 9 + 2 * g]  # A B C D
        E, F = g, 4 + g
        qcols = [64 * h + d for h in heads for d in range(32)] + \
                [64 * h + 32 + d for h in heads for d in range(32)]
        kcols = [64 * h + d for h in (E, F) for d in range(32)] + \
                [64 * h + 32 + d for h in (E, F) for d in range(32)]
        vcols = [64 * E + d for d in range(64)] + [64 * F + d for d in range(64)]
        prow = [64 * h + d for h in heads for d in range(64)]
        wgate_b = np.ascontiguousarray(
            np.concatenate([np.repeat(Wgate[:, E:E + 1], 64, 1),
                            np.repeat(Wgate[:, F:F + 1], 64, 1)], 1)).astype(BF)
        veT = np.ascontiguousarray(ve[b][:, vcols].T).astype(BF)
        maps.append({
            "xT": xT[b],
            "wq": np.ascontiguousarray(Wq[:, qcols]).astype(BF),
            "wk": np.ascontiguousarray(Wk[:, kcols]).astype(BF),
            "wv": np.ascontiguousarray(Wv[:, vcols]).astype(BF),
            "wgate": wgate_b,
            "wproj": np.ascontiguousarray(Wproj[prow, :]).astype(BF),
            "m1": m1, "mk": mk, "ident": ident,
            "veT": veT, "cos4": cos4, "sin4": sin4,
            "cmask": cmask, "wmask": wmask,
        })
    return maps


def _run(inputs, trace=False):
    from concourse.bass_utils import run_bass_kernel_spmd
    window = int(inputs["window"])
    assert window == 512, f"kernel tuned for window=512, got {window}"
    if window not in _CACHE:
        _CACHE[window] = _build(window)
    nc = _CACHE[window]
    maps = _host_inputs(inputs["x"], inputs["ve"], inputs["cos"], inputs["sin"],
                        inputs["Wq"], inputs["Wk"], inputs["Wv"],
                        inputs["Wproj"], inputs["Wgate"])
    res = run_bass_kernel_spmd(nc, maps, list(range(8)), trace=trace)
    y = np.zeros((B, T, C), dtype=np.float32)
    for core, r in enumerate(res.results):
        b = core // 4
        y[b] += r["outT"].T.astype(np.float32)
    return y, res


def kernel(**inputs):
    y, _ = _run(inputs, trace=False)
    return y


# revision 26
# speedup vs baseline: 1.0582x; 1.0582x over previous
"""Trainium2 Bass kernel for nn_CausalSelfAttention_72653666779352.

Sharding: 8 cores = 2 batches x 4 kv-groups. Core (b, g) owns
global kv head E=g (q heads 2g, 2g+1) and local kv head F=4+g
(q heads 8+2g, 9+2g). All device compute is in transposed layout
(feature dims on partitions, time on free axis), bf16 matmuls
(FWL weight loads + 1 cycle/row streaming; fp32r would disable FWL).
Scalar engine uses ONE activation table set (exp+ln): sigmoid is
computed as 1/(1+e^-x) and rsqrt as exp(-0.5*ln(x)) to avoid
ACT_TABLE_LOAD thrash. c_proj is fused into the attention loop
per q-chunk, reusing freed PSUM y-banks. Row-parallel partials
([C, T] bf16 per core) are summed on the host (unshard).
"""
import contextlib
import numpy as np
import ml_dtypes

BF = ml_dtypes.bfloat16
B, T, C = 2, 2048, 1024
NH, NKV = 16, 8
HD = 64
VGC = 32
TQC = 512            # tq chunk width
NQC = T // TQC       # 4
NKB = T // 128       # 16
EPS = float(np.finfo(np.float32).eps)
SCALE = 1.0 / 8.0    # 1/sqrt(HD)

_CACHE = {}


def _ranges(qc, pair, window):
    """kb tiles for (pair, qc): list of (kb, lo, hi, band_lo, mtype)."""
    out = []
    for kb in range(NKB):
        o = 128 * kb - TQC * qc
        lo = max(0, o)
        hi = TQC if pair == 0 else min(TQC, o + window + 128)
        if lo >= hi:
            continue
        cband = o if 0 <= o < TQC else None
        wband = None
        if pair == 1:
            wb = o + window
            if 0 <= wb < TQC:
                wband = wb
        assert not (cband is not None and wband is not None)
        if cband is not None:
            out.append((kb, lo, hi, cband, "c"))
        elif wband is not None:
            out.append((kb, lo, hi, wband, "w"))
        else:
            out.append((kb, lo, hi, None, None))
    # first tile must cover the full [0, TQC) col range (PSUM has_written)
    first = next(i for i, r in enumerate(out) if r[1] == 0 and r[2] == TQC)
    out[0], out[first] = out[first], out[0]
    return out


def _build(window):
    import concourse.mybir as mybir
    import concourse.tile as tile
    from concourse import bacc

    f32, bf16 = mybir.dt.float32, mybir.dt.bfloat16
    nc = bacc.Bacc("TRN2", target_bir_lowering=False, debug=False)

    def din(name, shape, dt=bf16):
        return nc.dram_tensor(name, shape, dt, kind="ExternalInput").ap()

    xT_d = din("xT", [C, T])
    wq_d = din("wq", [C, 256])
    wk_d = din("wk", [C, 128])
    wv_d = din("wv", [C, 128])
    wgate_d = din("wgate", [VGC, 128])
    wproj_d = din("wproj", [256, C])
    m1_d = din("m1", [128, 128])        # block-diag 32x32 ones (q rms sums)
    mk_d = din("mk", [128, 64])         # k rms indicator
    ident_d = din("ident", [128, 128])
    veT_d = din("veT", [128, T])        # 2*ve, per-core heads, transposed
    cos4_d = din("cos4", [128, T])
    sin4_d = din("sin4", [128, T])
    cmask_d = din("cmask", [128, 256])
    wmask_d = din("wmask", [128, 256])
    outT_d = nc.dram_tensor("outT", [C, T], bf16, kind="ExternalOutput").ap()

    EXP = mybir.ActivationFunctionType.Exp
    TANH = mybir.ActivationFunctionType.Tanh
    SQRT = mybir.ActivationFunctionType.Sqrt

    with tile.TileContext(nc) as tc, contextlib.ExitStack() as top:
        pers = top.enter_context(tc.tile_pool(name="pers", bufs=1))
        vap = top.enter_context(tc.tile_pool(name="vap", bufs=1))
        yTp = top.enter_context(tc.tile_pool(name="yTp", bufs=1))

        # ---- persistent loads (weights split per contraction chunk) ----
        wq_sb = pers.tile([128, 8, 256], bf16)
        wk_sb = pers.tile([128, 8, 128], bf16)
        wv_sb = pers.tile([128, 8, 128], bf16)
        wq_r = wq_d.rearrange("(a p) m -> p a m", p=128)
        wk_r = wk_d.rearrange("(a p) m -> p a m", p=128)
        wv_r = wv_d.rearrange("(a p) m -> p a m", p=128)
        for ct in range(8):
            nc.sync.dma_start(out=wq_sb[:, ct, :], in_=wq_r[:, ct, :])
            nc.sync.dma_start(out=wk_sb[:, ct, :], in_=wk_r[:, ct, :])
            nc.sync.dma_start(out=wv_sb[:, ct, :], in_=wv_r[:, ct, :])
        wgate_sb = pers.tile([VGC, 128], bf16)
        nc.sync.dma_start(out=wgate_sb, in_=wgate_d)
        eps_sb = pers.tile([128, 1], f32)
        nc.vector.memset(eps_sb, EPS)

        # persistent activations
        qf1 = pers.tile([128, T], bf16)   # [A | B] per-head normed q
        qf2 = pers.tile([128, T], bf16)   # [C | D]
        k_g = pers.tile([128, T], bf16)   # [E | E]
        k_l = pers.tile([128, T], bf16)   # [F | F]
        vT_sb = pers.tile([128, T], bf16)  # [E dims | F dims]
        veT_sb = pers.tile([128, T], bf16)
        nc.sync.dma_start(out=veT_sb, in_=veT_d)
        cos4_sb = pers.tile([128, T], bf16)
        nc.sync.dma_start(out=cos4_sb, in_=cos4_d)
        sin4_sb = pers.tile([128, T], bf16)
        nc.sync.dma_start(out=sin4_sb, in_=sin4_d)
        yT_g = yTp.tile([128, T], bf16)
        yT_l = yTp.tile([128, T], bf16)
        gth = pers.tile([128, T], bf16)   # tanh(g/2); gate = 1 + gth
        gx = pers.tile([VGC, T], bf16)    # x[:, 0:32] transposed, for gate
        nc.sync.dma_start(out=gx, in_=xT_d[0:VGC, :])

        # va tiles: [v(64) | ones(64)]; ones prefilled up-front on gpsimd
        va = {}
        for kb in range(NKB):
            for h in range(2):
                t = vap.tile([128, 128], bf16, name=f"va{h}_{kb}")
                nc.gpsimd.memset(t[:, 64:128], 1.0)
                va[(h, kb)] = t

        # deferred constant loads (behind the critical first-chunk path)
        m1_sb = pers.tile([128, 128], bf16)
        mk_sb = pers.tile([128, 64], bf16)
        ident_sb = pers.tile([128, 128], bf16)
        cmask_sb = pers.tile([128, 2, 128], bf16)
        wmask_sb = pers.tile([128, 2, 128], bf16)
        wp_sb = pers.tile([128, 2, C], bf16)

        # ---- phase 1: projections + rope + rmsnorm + gate/ve + v transp ----
        with contextlib.ExitStack() as ph1:
            p1s = ph1.enter_context(tc.tile_pool(name="p1s", bufs=2))
            psA = ph1.enter_context(tc.tile_pool(name="psA", bufs=5, space="PSUM"))
            psB = ph1.enter_context(tc.tile_pool(name="psB", bufs=2, space="PSUM"))
            psV = ph1.enter_context(tc.tile_pool(name="psV", bufs=1, space="PSUM"))

            xT_r = xT_d.rearrange("(a p) t -> p a t", p=128)
            xc0 = []
            for ct in range(8):
                xt = p1s.tile([128, TQC], bf16, tag=f"x{ct}", name=f"x{ct}_0", bufs=2)
                nc.sync.dma_start(out=xt, in_=xT_r[:, ct, 0:TQC])
                xc0.append(xt)

            nc.sync.dma_start(out=m1_sb, in_=m1_d)
            nc.sync.dma_start(out=mk_sb, in_=mk_d)
            nc.sync.dma_start(out=ident_sb, in_=ident_d)
            nc.sync.dma_start(out=cmask_sb, in_=cmask_d.rearrange("p (a c) -> p a c", a=2))
            nc.sync.dma_start(out=wmask_sb, in_=wmask_d.rearrange("p (a c) -> p a c", a=2))
            nc.sync.dma_start(out=wp_sb, in_=wproj_d.rearrange("(a p) m -> p a m", p=128))

            # gate for all chunks up-front: one tanh batch = one exp-set
            # table residency; phase-1 ACT then only needs sqrt+copy.
            for qc in range(NQC):
                gts = slice(qc * TQC, (qc + 1) * TQC)
                g_ps = psB.tile([128, TQC], f32, tag="rms", name=f"g_{qc}")
                nc.tensor.matmul(g_ps, wgate_sb, gx[:, gts], start=True, stop=True)
                nc.scalar.activation(gth[:, gts], g_ps, TANH, bias=0.0, scale=0.5)

            for qc in range(NQC):
                ts = slice(qc * TQC, (qc + 1) * TQC)
                if qc == 0:
                    xc = xc0
                else:
                    xc = []
                    for ct in range(8):
                        xt = p1s.tile([128, TQC], bf16, tag=f"x{ct}", name=f"x{ct}_{qc}",
                                      bufs=2)
                        nc.sync.dma_start(out=xt, in_=xT_r[:, ct, ts])
                        xc.append(xt)

                qlo_ps = psA.tile([128, TQC], f32, tag="pj", name=f"qlo_{qc}")
                qhi_ps = psA.tile([128, TQC], f32, tag="pj", name=f"qhi_{qc}")
                k_ps = psA.tile([128, TQC], f32, tag="pj", name=f"k_{qc}")
                v_ps = psA.tile([128, TQC], f32, tag="pj", name=f"v_{qc}")
                for ct in range(8):
                    st, sp = (ct == 0), (ct == 7)
                    nc.tensor.matmul(v_ps, wv_sb[:, ct, :], xc[ct], start=st, stop=sp)
                    nc.tensor.matmul(k_ps, wk_sb[:, ct, :], xc[ct], start=st, stop=sp)
                    nc.tensor.matmul(qlo_ps, wq_sb[:, ct, 0:128], xc[ct], start=st, stop=sp)
                    nc.tensor.matmul(qhi_ps, wq_sb[:, ct, 128:256], xc[ct], start=st, stop=sp)

                # gate: v += (1 + tanh(g/2)) * ve   (== v + 2*sigmoid(g)*ve)
                gv = p1s.tile([128, TQC], bf16, tag="gv", name=f"gv_{qc}")
                nc.vector.scalar_tensor_tensor(
                    gv, gth[:, ts], 1.0, veT_sb[:, ts],
                    op0=mybir.AluOpType.add, op1=mybir.AluOpType.mult)
                nc.vector.tensor_add(vT_sb[:, ts], v_ps, gv)

                # v transpose + va assembly for this chunk's 4 k-blocks
                for kb in range(4 * qc, 4 * qc + 4):
                    vt_ps = psV.tile([128, 128], bf16, tag="vt", name=f"vt_{kb}")
                    nc.tensor.transpose(vt_ps, vT_sb[:, kb * 128:(kb + 1) * 128],
                                        ident_sb)
                    for h in range(2):
                        nc.scalar.copy(va[(h, kb)][:, 0:64],
                                       vt_ps[:, h * 64:(h + 1) * 64])

                # evacuate q/k PSUM to bf16 (ACT) so rope runs in DVE 2x mode
                qloc = p1s.tile([128, TQC], bf16, tag="qloc", name=f"qloc_{qc}")
                qhic = p1s.tile([128, TQC], bf16, tag="qhic", name=f"qhic_{qc}")
                kc = p1s.tile([128, TQC], bf16, tag="kc", name=f"kc_{qc}")
                nc.scalar.copy(qloc, qlo_ps)
                nc.scalar.copy(qhic, qhi_ps)
                nc.scalar.copy(kc, k_ps)

                # q rope (split layout)
                mc = p1s.tile([128, TQC], bf16, tag="mc", name=f"mc_{qc}")
                ms = p1s.tile([128, TQC], bf16, tag="ms", name=f"ms_{qc}")
                rl = p1s.tile([128, TQC], bf16, tag="rl", name=f"rl_{qc}")
                rh = p1s.tile([128, TQC], bf16, tag="rh", name=f"rh_{qc}")
                nc.vector.tensor_mul(mc, qloc, cos4_sb[:, ts])
                nc.vector.tensor_mul(ms, qhic, sin4_sb[:, ts])
                nc.vector.tensor_add(rl, mc, ms)
                nc.vector.tensor_mul(mc, qhic, cos4_sb[:, ts])
                nc.vector.tensor_mul(ms, qloc, sin4_sb[:, ts])
                nc.vector.tensor_sub(rh, mc, ms)

                # k rope: kc rows [Elo|Flo|Ehi|Fhi] -> kr same row order
                kr = p1s.tile([128, TQC], bf16, tag="kr", name=f"kr_{qc}")
                mck = p1s.tile([64, TQC], bf16, tag="mck", name=f"mck_{qc}", bufs=1)
                msk = p1s.tile([64, TQC], bf16, tag="msk", name=f"msk_{qc}", bufs=1)
                nc.vector.tensor_mul(mck, kc[0:64, :], cos4_sb[0:64, ts])
                nc.vector.tensor_mul(msk, kc[64:128, :], sin4_sb[64:128, ts])
                nc.vector.tensor_add(kr[0:64, :], mck, msk)
                nc.vector.tensor_mul(mck, kc[64:128, :], cos4_sb[64:128, ts])
                nc.vector.tensor_mul(msk, kc[0:64, :], sin4_sb[0:64, ts])
                nc.vector.tensor_sub(kr[64:128, :], mck, msk)

                # rmsnorm q
                q2a = p1s.tile([128, TQC], bf16, tag="q2a", name=f"q2a_{qc}")
                q2b = p1s.tile([128, TQC], bf16, tag="q2b", name=f"q2b_{qc}")
                nc.vector.tensor_mul(q2a, rl, rl)
                nc.vector.tensor_mul(q2b, rh, rh)
                rms_q = psB.tile([128, TQC], f32, tag="rms", name=f"rmsq_{qc}")
                nc.tensor.matmul(rms_q, m1_sb, q2a, start=True, stop=False)
                nc.tensor.matmul(rms_q, m1_sb, q2b, start=False, stop=True)
                sq_sb = p1s.tile([128, TQC], f32, tag="sq", name=f"sq_{qc}")
                nc.scalar.activation(sq_sb, rms_q, SQRT, bias=eps_sb, scale=1.0 / HD)
                rq = p1s.tile([128, TQC], f32, tag="rq", name=f"rq_{qc}")
                nc.vector.reciprocal_approx_fast(rq, sq_sb)
                qn_lo = p1s.tile([128, TQC], bf16, tag="qnl", name=f"qnl_{qc}", bufs=2)
                qn_hi = p1s.tile([128, TQC], bf16, tag="qnh", name=f"qnh_{qc}", bufs=2)
                nc.vector.tensor_mul(qn_lo, rl, rq)
                nc.vector.tensor_mul(qn_hi, rh, rq)

                # rmsnorm k
                k2 = p1s.tile([128, TQC], bf16, tag="k2", name=f"k2_{qc}")
                nc.vector.tensor_mul(k2, kr, kr)
                rms_k = psB.tile([64, TQC], f32, tag="rms", name=f"rmsk_{qc}")
                nc.tensor.matmul(rms_k, mk_sb, k2, start=True, stop=True)
                sk_sb = p1s.tile([64, TQC], f32, tag="sk", name=f"sk_{qc}", bufs=1)
                nc.scalar.activation(sk_sb, rms_k, SQRT, bias=eps_sb[0:64, :], scale=1.0 / HD)
                rk = p1s.tile([64, TQC], f32, tag="rk", name=f"rk_{qc}", bufs=1)
                nc.vector.reciprocal_approx_fast(rk, sk_sb)
                rkd = p1s.tile([128, TQC], f32, tag="rkd", name=f"rkd_{qc}")
                nc.scalar.copy(rkd[0:64, :], rk)
                nc.scalar.copy(rkd[64:128, :], rk)
                kn = p1s.tile([128, TQC], bf16, tag="kn", name=f"kn_{qc}", bufs=2)
                nc.vector.tensor_mul(kn, kr, rkd)

                # permute split layout -> per-head tiles (SBUF->SBUF DMA)
                for i in range(4):
                    dst = qf1 if i < 2 else qf2
                    base = (i % 2) * 64
                    nc.sync.dma_start(out=dst[base:base + 32, ts],
                                      in_=qn_lo[i * 32:(i + 1) * 32, :])
                    nc.sync.dma_start(out=dst[base + 32:base + 64, ts],
                                      in_=qn_hi[i * 32:(i + 1) * 32, :])
                for half in range(2):
                    b0 = half * 64
                    nc.sync.dma_start(out=k_g[b0:b0 + 32, ts], in_=kn[0:32, :])
                    nc.sync.dma_start(out=k_g[b0 + 32:b0 + 64, ts], in_=kn[64:96, :])
                    nc.sync.dma_start(out=k_l[b0:b0 + 32, ts], in_=kn[32:64, :])
                    nc.sync.dma_start(out=k_l[b0 + 32:b0 + 64, ts], in_=kn[96:128, :])

        # ---- phase 3: attention + fused c_proj per q-chunk ----
        with contextlib.ExitStack() as ph3:
            psS = ph3.enter_context(tc.tile_pool(name="psS", bufs=1, space="PSUM"))
            psY = ph3.enter_context(tc.tile_pool(name="psY", bufs=2, space="PSUM"))
            wk3 = ph3.enter_context(tc.tile_pool(name="wk3", bufs=3))

            cfg = [(qf1, k_g, yT_g), (qf2, k_l, yT_l)]
            for qc in range(NQC):
                ts = slice(qc * TQC, (qc + 1) * TQC)
                rrs = [_ranges(qc, pair, window) for pair in range(2)]
                yps = [[psY.tile([128, TQC], f32, tag=f"y{pair}{h}", bufs=1,
                                 name=f"y{pair}{h}_{qc}")
                        for h in range(2)] for pair in range(2)]
                nmax = max(len(r) for r in rrs)
                for idx in range(nmax):
                    for pair in range(2):
                        if idx >= len(rrs[pair]):
                            continue
                        qf, kt, yT = cfg[pair]
                        kb, lo, hi, band, mt = rrs[pair][idx]
                        s2 = psS.tile([128, 2, TQC], f32, tag=f"s{pair}",
                                      name=f"s{pair}_{qc}_{kb}")
                        ks = slice(kb * 128, (kb + 1) * 128)
                        qs = slice(qc * TQC + lo, qc * TQC + hi)
                        nc.tensor.matmul(s2[:, 0, lo:hi], kt[0:64, ks], qf[0:64, qs],
                                         start=True, stop=True, tile_position=(0, 0))
                        nc.tensor.matmul(s2[:, 1, lo:hi], kt[64:128, ks], qf[64:128, qs],
                                         start=True, stop=True, tile_position=(64, 0))
                        st, sp = (idx == 0), (idx == len(rrs[pair]) - 1)
                        e2 = wk3.tile([128, 2, TQC], bf16, tag=f"e{pair}",
                                      name=f"e{pair}_{qc}_{kb}")
                        nc.scalar.activation(e2[:, :, lo:hi], s2[:, :, lo:hi], EXP,
                                             bias=0.0, scale=SCALE)
                        if band is not None:
                            msk_t = cmask_sb if mt == "c" else wmask_sb
                            nc.vector.tensor_mul(e2[:, :, band:band + 128],
                                                 e2[:, :, band:band + 128], msk_t)
                        for h in range(2):
                            nc.tensor.matmul(yps[pair][h][:, lo:hi], va[(pair, kb)],
                                             e2[:, h, lo:hi], start=st, stop=sp)
                for pair in range(2):
                    qf, kt, yT = cfg[pair]
                    for h in range(2):
                        y_ps = yps[pair][h]
                        den = wk3.tile([64, TQC], f32, tag=f"dn{pair}{h}",
                                       name=f"dn{pair}{h}_{qc}")
                        nc.vector.tensor_copy(den, y_ps[64:128, :])
                        rec = wk3.tile([64, TQC], f32, tag=f"rec{pair}{h}",
                                       name=f"rec{pair}{h}_{qc}")
                        nc.vector.reciprocal_approx_fast(rec, den)
                        nc.vector.tensor_mul(yT[h * 64:(h + 1) * 64, ts],
                                             y_ps[0:64, :], rec)

                # fused c_proj for this chunk (reuses pair-0 y banks)
                for cb in range(8):
                    cs = slice(cb * 128, (cb + 1) * 128)
                    pj = psY.tile([128, TQC], f32, tag=f"y0{cb % 2}", bufs=1,
                                  name=f"pj_{cb}_{qc}")
                    nc.tensor.matmul(pj, wp_sb[:, 0, cs], yT_g[:, ts],
                                     start=True, stop=False)
                    nc.tensor.matmul(pj, wp_sb[:, 1, cs], yT_l[:, ts],
                                     start=False, stop=True)
                    ot = wk3.tile([128, TQC], bf16, tag=f"ot{cb % 4}",
                                  name=f"ot_{cb}_{qc}")
                    if cb % 2 == 0:
                        nc.vector.tensor_copy(ot, pj)
                    else:
                        nc.scalar.copy(ot, pj)
                    nc.sync.dma_start(out=outT_d[cs, ts], in_=ot)

    nc.compile()
    return nc


def _host_inputs(x, ve, cos, sin, Wq, Wk, Wv, Wproj, Wgate):
    """Per-core input maps (core = b*4 + g)."""
    x = np.asarray(x, np.float32)
    ve = np.asarray(ve, np.float32)
    cos = np.asarray(cos, np.float32).reshape(T, -1)   # [T, 32]
    sin = np.asarray(sin, np.float32).reshape(T, -1)
    Wq = np.asarray(Wq, np.float32)
    Wk = np.asarray(Wk, np.float32)
    Wv = np.asarray(Wv, np.float32)
    Wproj = np.asarray(Wproj, np.float32)
    Wgate = np.asarray(Wgate, np.float32)

    cos4 = np.ascontiguousarray(np.tile(cos.T, (4, 1))).astype(BF)  # [128, T]
    sin4 = np.ascontiguousarray(np.tile(sin.T, (4, 1))).astype(BF)
    ident = np.eye(128, dtype=np.float32).astype(BF)
    m1 = np.kron(np.eye(4, dtype=np.float32), np.ones((32, 32), np.float32)).astype(BF)
    mk = np.zeros((128, 64), np.float32)
    for p in range(128):
        h = (p % 64) // 32
        mk[p, h * 32:(h + 1) * 32] = 1.0
    mk = mk.astype(BF)
    pp = np.arange(128)[:, None]
    cc = np.arange(128)[None, :]
    cmask = np.tile((cc >= pp).astype(np.float32), (1, 2)).reshape(128, 256).astype(BF)
    wmask = np.tile((cc <= pp).astype(np.float32), (1, 2)).reshape(128, 256).astype(BF)

    xT = [np.ascontiguousarray(x[b].T).astype(BF) for b in range(B)]
    maps = []
    for core in range(8):
        b, g = divmod(core, 4)
        heads = [2 * g, 2 * g + 1, 8 + 2 * g, 9 + 2 * g]  # A B C D
        E, F = g, 4 + g
        qcols = [64 * h + d for h in heads for d in range(32)] + \
                [64 * h + 32 + d for h in heads for d in range(32)]
        kcols = [64 * h + d for h in (E, F) for d in range(32)] + \
                [64 * h + 32 + d for h in (E, F) for d in range(32)]
        vcols = [64 * E + d for d in range(64)] + [64 * F + d for d in range(64)]
        prow = [64 * h + d for h in heads for d in range(64)]
        wgate_b = np.ascontiguousarray(
            np.concatenate([np.repeat(Wgate[:, E:E + 1], 64, 1),
                            np.repeat(Wgate[:, F:F + 1], 64, 1)], 1)).astype(BF)
        veT = np.ascontiguousarray(ve[b][:, vcols].T).astype(BF)
        maps.append({
            "xT": xT[b],
            "wq": np.ascontiguousarray(Wq[:, qcols]).astype(BF),
            "wk": np.ascontiguousarray(Wk[:, kcols]).astype(BF),
            "wv": np.ascontiguousarray(Wv[:, vcols]).astype(BF),
            "wgate": wgate_b,
            "wproj": np.ascontiguousarray(Wproj[prow, :]).astype(BF),
            "m1": m1, "mk": mk, "ident": ident,
            "veT": veT, "cos4": cos4, "sin4": sin4,
            "cmask": cmask, "wmask": wmask,
        })
    return maps


def _run(inputs, trace=False):
    from concourse.bass_utils import run_bass_kernel_spmd
    window = int(inputs["window"])
    assert window == 512, f"kernel tuned for window=512, got {window}"
    if window not in _CACHE:
        _CACHE[window] = _build(window)
    nc = _CACHE[window]
    maps = _host_inputs(inputs["x"], inputs["ve"], inputs["cos"], inputs["sin"],
                        inputs["Wq"], inputs["Wk"], inputs["Wv"],
                        inputs["Wproj"], inputs["Wgate"])
    res = run_bass_kernel_spmd(nc, maps, list(range(8)), trace=trace)
    y = np.zeros((B, T, C), dtype=np.float32)
    for core, r in enumerate(res.results):
        b = core // 4
        y[b] += r["outT"].T.astype(np.float32)
    return y, res


def kernel(**inputs):
    y, _ = _run(inputs, trace=False)
    return y
